# revision 34
# baseline (speedup 1.0000x reference)
"""Biquad lowpass filter (torchaudio lowpass_biquad, SR=24000, cutoff=8000, Q=0.707)
over wav [64, 480000], data-parallel across 8 TRN2 NeuronCores.

The biquad's poles sit at |z| = sqrt(a2) ~= 0.49, so the IIR is numerically an
8-tap causal FIR at the 2e-2 tolerance (tail energy beyond tap 8 is 5.8e-3).
The kernel boundary is bf16: the host rounds wav to bf16 before upload and
upcasts the bf16 result, halving HBM traffic (15.4 MB/core, ~44 us DMA floor).

Layout per core: 8 rows x 480000 = 128 chunks of 30000 samples, one per SBUF
partition. Time is cut into 120-sample slices; each slice's 128-sample window
(8-sample halo + 120 new) is PE-transposed so window-time sits on partitions,
then one independent start=stop=True matmul against the banded H' [128, 120]
(H'[w, n] = h[n+8-w]) produces the slice's output. No carry matmuls, no PSUM
accumulation overlap: chunk continuity is the DMA halo (8 samples re-read
before each group; zeros at row starts, patched exactly on the host). DVE does
every PSUM->SBUF slab copy (bf16 2x mode, on the PE-critical chain), Act every
f32->bf16 output drain. Input rides the sync (SP) HWDGE ring in 32-slice
groups; mid-stream output flushes ride gpsimd SWDGE, tail flushes the then-idle
sync ring, per sub-iteration. Constants ([Hp | I]) are built on-device via
affine_select. Emission is software-pipelined two sub-iterations ahead so the
PE never waits on a slab copy. Measured 57.4 us on HW (DMA-pool floor ~43 us
for 15.4 MB/core, plus ~7 us fixed prologue and ~9 us semaphore-cleanup
epilogue).
"""

import sys

sys.path.insert(0, "/opt/trn_rl_repo")

import numpy as np
import ml_dtypes

import concourse.mybir as mybir
import concourse.tile as tile
from concourse import bacc
from concourse.bass_utils import run_bass_kernel_spmd

f32 = mybir.dt.float32
bf16 = mybir.dt.bfloat16

# ---- problem constants ----------------------------------------------------
SR = 24000
CUTOFF = 8000.0
Q = 0.707

B_FULL, T = 64, 480000
N_CORES = 8
R = B_FULL // N_CORES          # rows per core
NCH = 16                       # chunks per row
P = R * NCH                    # 128 partitions (one chunk each)
L = T // NCH                   # 30000 samples per chunk
LS = 120                       # slice length
D = 8                          # FIR taps (window = LS + D = 128 partitions)
W = LS + D                     # 128: transpose window rows
NSL = L // LS                  # 250 slices per chunk
SUB = 4                        # slices per PSUM sub-iteration (one bank)


def _group_plan():
    """Slices per DMA group: small first group (compute starts early), then
    32-wide (few stream boundaries, 7.7KB descriptors), moderate last group.
    Sums to NSL."""
    plan = [8] + [32] * 7 + [18]
    assert sum(plan) == NSL, sum(plan)
    return plan


def _sub_plan(gi, nsl):
    """Sub-iteration sizes within a group; group 0 leads with a 2-slice sub so
    the first matmul only waits on a quarter-size transfer."""
    if gi == 0:
        sizes = [2, 4, 2]
        assert sum(sizes) == nsl
        return sizes
    sizes = []
    left = nsl
    while left:
        s = min(SUB, left)
        sizes.append(s)
        left -= s
    return sizes


GS_MAX = 32                    # widest group (tile allocation size)
GW = GS_MAX * LS


def _fir_taps():
    w0 = 2.0 * np.pi * CUTOFF / SR
    alpha = np.sin(w0) / (2.0 * Q)
    cos_w0 = np.cos(w0)
    b0 = (1.0 - cos_w0) / 2.0
    b1 = 1.0 - cos_w0
    b2 = b0
    a0 = 1.0 + alpha
    a1 = -2.0 * cos_w0
    a2 = 1.0 - alpha
    b0, b1, b2, a1, a2 = (np.float32(b0 / a0), np.float32(b1 / a0),
                          np.float32(b2 / a0), np.float32(a1 / a0),
                          np.float32(a2 / a0))
    h = np.zeros(D, dtype=np.float64)
    x1 = x2 = y1 = y2 = 0.0
    for t in range(D):
        x = 1.0 if t == 0 else 0.0
        y = (float(b0) * x + float(b1) * x1 + float(b2) * x2
             - float(a1) * y1 - float(a2) * y2)
        h[t] = y
        x2, x1 = x1, x
        y2, y1 = y1, y
    return h


def _emit_const_blk(nc, cblk):
    """Build [Hp | ident] on-device (no inline-tensor upload): memset, then one
    affine_select per FIR diagonal (Hp[w, n] = h[n + D - w], i.e. the band
    w - n + d - D == 0 gets fill h[d]), then the identity diagonal."""
    h = _fir_taps()
    nc.gpsimd.memset(cblk[:], 0.0)
    Hp = cblk[:, 0:LS]
    ident = cblk[:, LS:]
    for dd in range(D):
        nc.gpsimd.affine_select(
            out=Hp, in_=Hp,
            compare_op=mybir.AluOpType.not_equal,
            fill=float(h[dd]),
            base=dd - D,
            pattern=[[-1, LS]],
            channel_multiplier=1,
        )
    nc.gpsimd.affine_select(
        out=ident, in_=ident,
        compare_op=mybir.AluOpType.not_equal,
        fill=1.0,
        base=0,
        pattern=[[-1, W]],
        channel_multiplier=1,
    )


def _build():
    nc = bacc.Bacc("TRN2", target_bir_lowering=False)

    wav = nc.dram_tensor("wav", [R, T], bf16, kind="ExternalInput")
    out = nc.dram_tensor("out", [R, T], bf16, kind="ExternalOutput")

    wav_ch = wav[:, :].rearrange("r (c l) -> (r c) l", c=NCH)   # [128, 30000]
    out_ch = out[:, :].rearrange("r (c l) -> (r c) l", c=NCH)

    groups = []
    done = 0
    for n in _group_plan():
        groups.append((done, n))
        done += n
    n_groups = len(groups)

    with tile.TileContext(nc) as tc:
        with (
            tc.tile_pool(name="const", bufs=1) as cpool,
            tc.tile_pool(name="io", bufs=6) as iopool,
            tc.tile_pool(name="work", bufs=4) as wpool,
            tc.tile_pool(name="psum", bufs=4, space="PSUM") as ppool,
        ):
            cblk = cpool.tile([W, LS + W], bf16)
            _emit_const_blk(nc, cblk)
            Hp = cblk[:, 0:LS]
            ident = cblk[:, LS:]

            # gpsimd cannot touch PSUM, so only Act and DVE can copy. Static
            # split: DVE takes every slab copy (bf16->bf16 runs in its 2x
            # mode, ~0.4us) — that's the PE-critical chain — and Act takes
            # every drain (f32 PSUM -> bf16, ~0.6us), which is off-path.
            slab_copy = lambda o, i: nc.vector.tensor_copy(o, i)
            drain_copy = lambda o, i: nc.scalar.copy(o, i)

            # software pipeline, 2 stages deep: sub-iter i's matmuls + drain
            # are emitted after sub-iter i+2's transposes, so the slab-copy
            # chain (transpose -> DVE copy -> matmul) has two pipeline
            # periods of slack and never stalls the PE.
            pend = []
            sub_idx = 0

            def emit_tail(pend, drain=None):
                slab, s, si, yout, flush = pend
                drain = drain or drain_copy
                py = ppool.tile([P, SUB * LS], f32, tag="py")
                for j in range(s):
                    nc.tensor.matmul(
                        py[:, j * LS: (j + 1) * LS],
                        slab[:, j * P: (j + 1) * P],
                        Hp,
                        start=True, stop=True, skip_group_check=True,
                    )
                drain(yout[:, si * LS: (si + s) * LS], py[:, : s * LS])
                if flush is not None:
                    gbase, a, b, yt, eng = flush
                    eng.dma_start(out_ch[:, gbase + a: gbase + b], yt[:, a:b])

            for gi, (sl0, nsl) in enumerate(groups):
                gbase = sl0 * LS
                gw = nsl * LS

                xin = iopool.tile([P, D + GW], bf16, tag="xin")
                if gi == 0:
                    # First group: the halo (= previous chunk's last D samples)
                    # rides a partition-shifted DMA ahead of the input on the
                    # sync ring; group-0's input is split so the first
                    # sub-iteration can start after a quarter transfer.
                    # Row-start partitions get the previous row's tail —
                    # finite but wrong; their first D output samples are
                    # recomputed exactly on the host. Partition 0's halo is
                    # memset (uninitialized SBUF could hold NaN, and NaN*0
                    # poisons the matmul).
                    nc.sync.dma_start(xin[1:P, 0:D], wav_ch[0: P - 1, L - D: L])
                    h1 = 2 * LS
                    nc.sync.dma_start(xin[:, D: D + h1], wav_ch[:, 0:h1])
                    nc.sync.dma_start(xin[:, D + h1: D + gw], wav_ch[:, h1:gw])
                    nc.gpsimd.memset(xin[0:1, 0:D], 0.0)
                elif gi == 1:
                    # split: group 0 is small, so the PE runs dry waiting for
                    # group 1's full 3.9us transfer — landing the first half
                    # early bridges the pipeline fill
                    h1 = (nsl // 2) * LS
                    nc.sync.dma_start(
                        xin[:, 0: D + h1], wav_ch[:, gbase - D: gbase + h1])
                    nc.sync.dma_start(
                        xin[:, D + h1: D + gw], wav_ch[:, gbase + h1: gbase + gw])
                else:
                    nc.sync.dma_start(
                        xin[:, 0: D + gw],
                        wav_ch[:, gbase - D: gbase + gw],
                    )
                yout = iopool.tile([P, GW], bf16, tag="yout")

                last_group = gi == n_groups - 1
                si = 0
                for s in _sub_plan(gi, nsl):
                    pt = ppool.tile([W, SUB * P], bf16, tag="pt")
                    for j in range(s):
                        k = si + j
                        nc.tensor.transpose(
                            pt[:, j * P: (j + 1) * P],
                            xin[:, k * LS: k * LS + W],
                            ident,
                        )
                    slab = wpool.tile([W, SUB * P], bf16, tag="slab")
                    slab_copy(slab[:, : s * P], pt[:, : s * P])

                    if len(pend) >= 2:
                        emit_tail(pend.pop(0))
                    # Tail groups flush on the sync ring (input is finished by
                    # then and HWDGE dispatch is much faster than SWDGE gen on
                    # Pool); earlier groups flush whole on gpsimd, overlapped
                    # with the input stream.
                    if last_group:
                        # flush each sub-iteration as soon as it drains — the
                        # tail DMA is never one exposed burst
                        flush = (gbase, si * LS, (si + s) * LS, yout, nc.sync)
                    elif si + s >= nsl:
                        eng = nc.sync if gi == n_groups - 2 else nc.gpsimd
                        flush = (gbase, 0, gw, yout, eng)
                    else:
                        flush = None
                    pend.append((slab, s, si, yout, flush))
                    sub_idx += 1
                    si += s

            # the two leftover drains would serialize on Act — send the very
            # last one to the then-idle DVE instead
            for i, t in enumerate(pend):
                emit_tail(t, drain=slab_copy if i == len(pend) - 1 else None)

    nc.finalize()
    return nc


def _patch_warmup(out: np.ndarray, wav: np.ndarray):
    """Each waveform's first D samples start from zero filter state; the
    device computed them with a bogus halo. Run the exact IIR recurrence for
    those D samples on the host."""
    w0 = 2.0 * np.pi * CUTOFF / SR
    alpha = np.sin(w0) / (2.0 * Q)
    cos_w0 = np.cos(w0)
    a0 = 1.0 + alpha
    b0 = np.float32((1.0 - cos_w0) / 2.0 / a0)
    b1 = np.float32((1.0 - cos_w0) / a0)
    b2 = np.float32((1.0 - cos_w0) / 2.0 / a0)
    a1 = np.float32(-2.0 * cos_w0 / a0)
    a2 = np.float32((1.0 - alpha) / a0)
    x = wav[:, :D].astype(np.float64)
    B = x.shape[0]
    x1 = np.zeros(B); x2 = np.zeros(B)
    y1 = np.zeros(B); y2 = np.zeros(B)
    for t in range(D):
        xt = x[:, t]
        yt = b0 * xt + b1 * x1 + b2 * x2 - a1 * y1 - a2 * y2
        out[:, t] = yt.astype(np.float32)
        x2, x1 = x1, xt
        y2, y1 = y1, yt


_NC_CACHE = None


def _get_nc():
    global _NC_CACHE
    if _NC_CACHE is None:
        _NC_CACHE = _build()
    return _NC_CACHE


def _run(wav_full: np.ndarray, trace: bool = False):
    global _NC_CACHE
    wav_full = np.ascontiguousarray(wav_full, dtype=np.float32)
    wav_bf = wav_full.astype(ml_dtypes.bfloat16)
    in_maps = [
        {"wav": wav_bf[i * R: (i + 1) * R]} for i in range(N_CORES)
    ]
    last_err = None
    for attempt in range(3):
        try:
            res = run_bass_kernel_spmd(
                _get_nc(), in_maps, core_ids=list(range(N_CORES)), trace=trace
            )
            out = np.concatenate(
                [np.asarray(res.results[i]["out"]) for i in range(N_CORES)],
                axis=0).astype(np.float32)
            _patch_warmup(out, wav_full)
            return out, res
        except Exception as e:          # transient device errors recover on retry
            last_err = e
            _NC_CACHE = None
            try:
                import jax
                jax.clear_caches()
            except Exception:
                pass
            import time
            time.sleep(5 * (attempt + 1))
    raise last_err


def kernel(wav: np.ndarray) -> np.ndarray:
    out, _ = _run(np.asarray(wav))
    return out
